# revision 1
# baseline (speedup 1.0000x reference)
"""BitLinear TRN2 kernel: out = layernorm(x) @ sign(w).T + bias.

Tensor-parallel over out_features, 8 cores: each core gets full x
[8192, 4096] + a [2048, 4096] shard of w (+ bias shard); returns the
[8192, 2048] out shard; host concats.

LN is folded around a matmul on RAW x:
    out[t,o] = (x@bw.T)[t,o] * inv_t + a_t * S[o] + bias[o]
with S[o] = sum_d bw[o,d], inv_t = 1/(std_t+eps), a_t = -mu_t*inv_t.
The rank-1 correction + bias ride the PSUM->SBUF eviction as DVE ops
against broadcast-resident S/bias rows. Stats come from bn_stats on the
natural-layout x tiles; x and sign(w) are transposed on-chip by PE
transposes (contraction dim must sit on partitions); transposed sign
weights bounce through DRAM once (one tensor per 256-wide out chunk so
every DMA is a trivial contiguous 2D pattern).

Engine discipline (walrus: fp32/f32r matmuls+transposes get ONE
semaphore-wait slot; complex multi-dim DMA APs also overflow wait
slots): sign(w) is cast to bf16 so W transposes ride the multi-wait bf16
path; every DMA feeding f32r matmuls is first "touched" by a throwaway
bf16 bitcast-transpose so the PE sequencer observes its semaphore; all
psum evictions/copies ride DVE only. Every fp32-family PE instruction
then needs at most one wait (the DVE clock).

Modes: f32r (1 cyc/row, ~2e-4 rel err), split (bf16 hi+lo, 2 matmuls,
~3e-6), bf16 (1 matmul, ~2e-3).
"""

import os
from contextlib import ExitStack

import numpy as np

import concourse.bass as bass
import concourse.tile as tile
from concourse import bacc
from concourse import mybir
from concourse.bass_utils import run_bass_kernel_spmd
from concourse.masks import make_identity
from concourse.tile_rust import add_dep_helper

F32 = mybir.dt.float32
F32R = mybir.dt.float32r
BF16 = mybir.dt.bfloat16

T, D, O_FULL, NCORES = 8192, 4096, 16384, 8
O = O_FULL // NCORES  # 2048 out-features per core
EPS = 1e-5

MODE = os.environ.get("BITLIN_MODE", "f32r")  # f32r | split | bf16

KT = D // 128  # 32 k-tiles
MC = T // 128  # 64 token chunks
SUP = 512  # tokens per superblock (resident transposed-x width)
MPS = SUP // 128  # 4 chunks per superblock
NSUP = T // SUP  # 16
KB = 8  # k-tiles per weight-prep write batch
WN = 256  # moving free width (f32r needs >=256; one PSUM bank at fp32)
NWCH = O // WN  # 8
OBP = WN // 128  # o-blocks per out chunk


def _build(mode):
    wdt = F32R if mode == "f32r" else BF16
    xdt = F32R if mode == "f32r" else BF16

    nc = bacc.Bacc("TRN2", target_bir_lowering=False, debug=False)
    x_ext = nc.declare_dram_parameter("x", [T, D], F32, isOutput=False)
    w_ext = nc.declare_dram_parameter("w", [O, D], F32, isOutput=False)
    b_ext = nc.declare_dram_parameter("b", [O], F32, isOutput=False)
    out_ext = nc.declare_dram_parameter("out", [T, O], F32, isOutput=True)
    wtq = [nc.dram_tensor(f"wtq{nw}", [128, KT, WN], wdt) for nw in range(NWCH)]
    s_d = [nc.dram_tensor(f"srow{nw}", [WN], F32) for nw in range(NWCH)]

    with tile.TileContext(nc) as tc, ExitStack() as ctx:
        singles = ctx.enter_context(tc.tile_pool(name="singles", bufs=1))
        xa_pool = ctx.enter_context(tc.tile_pool(name="xa", bufs=2))
        ws_pool = ctx.enter_context(tc.tile_pool(name="ws", bufs=1))
        xt_pool = ctx.enter_context(tc.tile_pool(name="xt", bufs=1))
        wst_pool = ctx.enter_context(tc.tile_pool(name="wst", bufs=2))
        wsb_pool = ctx.enter_context(tc.tile_pool(name="wsb", bufs=1))
        ev_pool = ctx.enter_context(tc.tile_pool(name="ev", bufs=2))
        evt_pool = ctx.enter_context(tc.tile_pool(name="evt", bufs=2))
        small = ctx.enter_context(tc.tile_pool(name="small", bufs=4))
        tmp_pool = ctx.enter_context(tc.tile_pool(name="tmp", bufs=2))
        tp_psum = ctx.enter_context(tc.tile_pool(name="tp_ps", bufs=3, space="PSUM"))
        tb_psum = ctx.enter_context(tc.tile_pool(name="tb_ps", bufs=1, space="PSUM"))
        mm_psum = ctx.enter_context(tc.tile_pool(name="mm_ps", bufs=3, space="PSUM"))
        s_psum = ctx.enter_context(tc.tile_pool(name="s_ps", bufs=1, space="PSUM"))

        identity = singles.tile([128, 128], F32)
        make_identity(nc, identity[:])
        identity_b = singles.tile([128, 128], BF16)
        nc.vector.tensor_copy(out=identity_b[:], in_=identity[:])
        ones32 = singles.tile([128, 1], F32)
        nc.vector.memset(ones32[:], 1.0)
        ones_w = singles.tile([128, 1], wdt)
        nc.vector.tensor_copy(out=ones_w[:], in_=ones32[:])
        inv_all = singles.tile([128, MC], F32)
        a_all = singles.tile([128, MC], F32)
        s_bc = singles.tile([128, NWCH, WN], BF16 if mode == "f32r" else F32)  # S bcast
        b_bc = singles.tile([128, NWCH, WN], F32)  # bias broadcast

        def touch(src_ap):
            """Throwaway bf16 transpose reading src so PE observes its sem."""
            pt = tb_psum.tile([128, 128], BF16, tag="tb")
            nc.tensor.transpose(pt[:], src_ap, identity_b[:])

        # bias broadcast (partition-stride-0 SWDGE dma)
        bap = b_ext[:]
        nc.gpsimd.dma_start(
            out=b_bc[:],
            in_=bass.AP(tensor=bap.tensor, offset=bap.offset, ap=[[0, 128]] + bap.ap),
        )

        # ------- weight prep: sign (bf16) + transpose -> per-chunk DRAM ---
        w_last_write = [None] * NWCH
        for nw in range(NWCH):
            ws_tiles = []
            for obl in range(OBP):
                ob = nw * OBP + obl
                wa = xa_pool.tile([128, D], F32, tag="xa")
                nc.gpsimd.dma_start(out=wa[:], in_=w_ext[ob * 128 : (ob + 1) * 128, :])
                ws = ws_pool.tile([128, D], BF16, tag=f"ws{obl}")
                nc.scalar.sign(out=ws[:], in_=wa[:])
                ws_tiles.append(ws)
            for kb in range(KT // KB):
                wt_sb = wsb_pool.tile([128, KB, WN], wdt, tag="wt_sb")
                for ki in range(KB):
                    k = kb * KB + ki
                    for obl in range(OBP):
                        pt = tp_psum.tile([128, 128], BF16, tag="tp")
                        nc.tensor.transpose(
                            pt[:],
                            ws_tiles[obl][:, k * 128 : (k + 1) * 128],
                            identity_b[:],
                        )
                        nc.vector.tensor_copy(
                            out=wt_sb[:, ki, obl * 128 : (obl + 1) * 128], in_=pt[:]
                        )
                inst = nc.gpsimd.dma_start(
                    out=wtq[nw][:, kb * KB : (kb + 1) * KB, :], in_=wt_sb[:]
                )
                if w_last_write[nw] is not None:
                    add_dep_helper(
                        inst.ins, w_last_write[nw].ins, sync=True, reason="wtq order"
                    )
                w_last_write[nw] = inst

        # ---------------- main: per token-superblock ---------------------
        for s in range(NSUP):
            xt = xt_pool.tile([128, KT, SUP], xdt, tag="xt_hi")
            xt_lo = (
                xt_pool.tile([128, KT, SUP], BF16, tag="xt_lo")
                if mode == "split"
                else None
            )

            for mc_i in range(MPS):
                m = s * MPS + mc_i
                xa = xa_pool.tile([128, D], F32, tag="xa")
                nc.gpsimd.dma_start(out=xa[:], in_=x_ext[m * 128 : (m + 1) * 128, :])
                touch(xa[:].bitcast(BF16)[:, 0:128])
                # --- stats ---
                st = small.tile([128, 8, 6], F32, tag="st")
                for j in range(8):
                    nc.vector.bn_stats(
                        out=st[:, j, :], in_=xa[:, j * 512 : (j + 1) * 512]
                    )
                mv = small.tile([128, 2], F32, tag="mv")
                nc.vector.bn_aggr(out=mv[:], in_=st[:])
                sc = small.tile([128, 2], F32, tag="sc")  # [negmu, den]
                nc.scalar.activation(
                    out=sc[:, 1:2],
                    in_=mv[:, 1:2],
                    func=mybir.ActivationFunctionType.Sqrt,
                    scale=float(D) / float(D - 1),
                )
                nc.vector.tensor_scalar_add(sc[:, 1:2], sc[:, 1:2], EPS)
                nc.vector.tensor_scalar_mul(sc[:, 0:1], mv[:, 0:1], -1.0)
                nc.vector.reciprocal(out=inv_all[:, m : m + 1], in_=sc[:, 1:2])
                nc.vector.tensor_mul(
                    a_all[:, m : m + 1], sc[:, 0:1], inv_all[:, m : m + 1]
                )
                # --- transpose x chunk (fp32 PE transposes; 1 DVE wait each) ---
                for k in range(KT):
                    pt = tp_psum.tile([128, 128], F32, tag="tp")
                    nc.tensor.transpose(
                        pt[:], xa[:, k * 128 : (k + 1) * 128], identity[:]
                    )
                    dst = xt[:, k, mc_i * 128 : (mc_i + 1) * 128]
                    nc.vector.tensor_copy(out=dst, in_=pt[:])
                    if mode == "split":
                        hi32 = tmp_pool.tile([128, 128], F32, tag="hi32")
                        nc.vector.tensor_copy(out=hi32[:], in_=dst)
                        nc.vector.tensor_sub(
                            xt_lo[:, k, mc_i * 128 : (mc_i + 1) * 128],
                            pt[:],
                            hi32[:],
                        )

            # --- matmuls against streamed transposed weights ---
            for nw in range(NWCH):
                wt = wst_pool.tile([128, KT, WN], wdt, tag="wst")
                rd = nc.gpsimd.dma_start(out=wt[:], in_=wtq[nw][:, :, :])
                add_dep_helper(
                    rd.ins, w_last_write[nw].ins, sync=True, reason="wtq RAW"
                )
                if mode == "f32r":
                    touch(wt[:, 0, :].bitcast(BF16)[:, 0:128])
                else:
                    touch(wt[:, 0, 0:128])
                if s == 0:
                    # S row: ones-matmul, stage out, bounce via DRAM, broadcast
                    ps_s = s_psum.tile([1, WN], F32, tag="s_ps")
                    for k in range(KT):
                        nc.tensor.matmul(
                            ps_s[:],
                            ones_w[:],
                            wt[:, k, :],
                            start=(k == 0),
                            stop=(k == KT - 1),
                        )
                    sstage = singles.tile([1, WN], F32, tag="sstage")
                    nc.vector.tensor_copy(out=sstage[:], in_=ps_s[:])
                    wr = nc.gpsimd.dma_start(out=s_d[nw][:], in_=sstage[:])
                    sap = s_d[nw][:]
                    br = nc.gpsimd.dma_start(
                        out=s_bc[:, nw, :],
                        in_=bass.AP(
                            tensor=sap.tensor, offset=sap.offset, ap=[[0, 128]] + sap.ap
                        ),
                    )
                    add_dep_helper(br.ins, wr.ins, sync=True, reason="Srow RAW")
                for mc_i in range(MPS):
                    m = s * MPS + mc_i
                    pm = mm_psum.tile([128, WN], F32, tag="mm")
                    tsl = slice(mc_i * 128, (mc_i + 1) * 128)
                    for k in range(KT):
                        nc.tensor.matmul(
                            pm[:],
                            xt[:, k, tsl],
                            wt[:, k, :],
                            start=(k == 0),
                            stop=(k == KT - 1 and xt_lo is None),
                        )
                        if xt_lo is not None:
                            nc.tensor.matmul(
                                pm[:],
                                xt_lo[:, k, tsl],
                                wt[:, k, :],
                                start=False,
                                stop=(k == KT - 1),
                            )
                    # evict: ev = pm*inv + a*S + bias  (all DVE)
                    tv = evt_pool.tile([128, WN], F32, tag="tv")
                    nc.vector.tensor_scalar_mul(
                        tv[:], s_bc[:, nw, :], a_all[:, m : m + 1]
                    )
                    ev = ev_pool.tile([128, WN], F32, tag="ev")
                    nc.vector.tensor_scalar_mul(ev[:], pm[:], inv_all[:, m : m + 1])
                    nc.vector.tensor_add(ev[:], ev[:], tv[:])
                    nc.vector.tensor_add(ev[:], ev[:], b_bc[:, nw, :])
                    nc.gpsimd.dma_start(
                        out=out_ext[m * 128 : (m + 1) * 128, nw * WN : (nw + 1) * WN],
                        in_=ev[:],
                    )
    nc.compile()
    return nc


_NC_CACHE = {}
LAST_RESULTS = None


def kernel(x, weight, bias):
    global LAST_RESULTS
    x = np.ascontiguousarray(np.asarray(x, dtype=np.float32))
    weight = np.asarray(weight, dtype=np.float32)
    bias = np.asarray(bias, dtype=np.float32)

    mode = MODE
    if mode not in _NC_CACHE:
        _NC_CACHE[mode] = _build(mode)
    nc = _NC_CACHE[mode]

    in_maps = []
    for i in range(NCORES):
        in_maps.append(
            {
                "x": x,
                "w": np.ascontiguousarray(weight[i * O : (i + 1) * O]),
                "b": np.ascontiguousarray(bias[i * O : (i + 1) * O]),
            }
        )
    trace = os.environ.get("BITLIN_TRACE", "0") == "1"
    try:
        res = run_bass_kernel_spmd(nc, in_maps, list(range(NCORES)), trace=trace)
    except Exception:
        if not trace:
            raise
        res = run_bass_kernel_spmd(nc, in_maps, list(range(NCORES)), trace=False)
    LAST_RESULTS = res
    return np.concatenate([res.results[i]["out"] for i in range(NCORES)], axis=1)



# revision 2
# speedup vs baseline: 2.0798x; 2.0798x over previous
"""BitLinear TRN2 kernel: out = layernorm(x) @ sign(w).T + bias.

Tensor-parallel over out_features, 8 cores: each core gets x^T (bf16,
host-transposed) + a [4096, 2048] transposed sign-weight shard (fp8,
host-prepped); returns the [8192, 2048] out shard (bf16); host concats
and casts to f32.

All layout/elementwise prep rides the host (sign, transposes, bf16/fp8
casts, LN stats); the device does the O(T*D*O) einsum at the PE bf16
roofline plus a 2-op DVE evict. LN is folded around the matmul on raw
x:  out[t,o] = (x@bw.T)[t,o] * inv_t + a_t * S[o] + bias[o]
with S[o] = sum_d bw[o,d], inv_t = 1/(std_t+eps), a_t = -mu_t*inv_t.
inv/a arrive per-token from the host arranged [128, 64] (partition =
token%128, col = token chunk); a_t*S[o]+bias[o] is built once per token
chunk (cb), and each PSUM eviction is psum*inv (+cb) straight to bf16.

Device layout: transposed sign weights live resident in SBUF as bf16
[128, 32, 2048] (128KB/partition), cast once from the fp8 shipment.
x^T streams in 256-token superblocks [128, 32, 256] bf16 (one 3D-AP DMA
each). Matmuls: stationary = x^T k-tile [128, 128], moving = weight
slice [128, 512] (one fp32 PSUM bank), 32-deep k accumulation; 8192 MMs
total per core ~= the 78.6 TF/s bf16 roofline. No PE transposes, no
on-device stats, no f32r single-wait hazards (everything PE-side is
bf16).

Output is written bf16 (rel err ~2e-3 << 2e-2 gate) to halve the
device->host return path; host casts back to f32.
"""

import os

import numpy as np
import ml_dtypes

import concourse.bass as bass
import concourse.tile as tile
from concourse import bacc
from concourse import mybir
from concourse.bass_utils import run_bass_kernel_spmd

F32 = mybir.dt.float32
BF16 = mybir.dt.bfloat16
FP8 = mybir.dt.float8e4

NP_BF16 = ml_dtypes.bfloat16
NP_FP8 = ml_dtypes.float8_e4m3

T, D, O_FULL, NCORES = 8192, 4096, 16384, 8
O = O_FULL // NCORES  # 2048 out-features per core
EPS = 1e-5

KT = D // 128  # 32 k-tiles
MC = T // 128  # 64 token chunks
SUP = 256  # tokens per superblock (resident transposed-x width)
CPS = SUP // 128  # 2 chunks per superblock
NSUP = T // SUP  # 32
WN = 512  # moving free width (one PSUM bank at fp32)
NWCH = O // WN  # 4 out chunks

MODE = os.environ.get("BITLIN_MODE", "bf16_hostprep")


def _build():
    nc = bacc.Bacc("TRN2", target_bir_lowering=False, debug=False)
    xt_d = nc.declare_dram_parameter("xt", [D, T], BF16, isOutput=False)
    w8_d = nc.declare_dram_parameter("w8", [D, O], FP8, isOutput=False)
    s_d = nc.declare_dram_parameter("srow", [O], BF16, isOutput=False)
    b_d = nc.declare_dram_parameter("brow", [O], BF16, isOutput=False)
    iv_d = nc.declare_dram_parameter("iv", [128, MC], F32, isOutput=False)
    aa_d = nc.declare_dram_parameter("aa", [128, MC], F32, isOutput=False)
    out_d = nc.declare_dram_parameter("out", [T, O], BF16, isOutput=True)

    with tile.TileContext(nc) as tc:
        with (
            tc.tile_pool(name="singles", bufs=1) as singles,
            tc.tile_pool(name="w8p", bufs=2) as w8p,
            tc.tile_pool(name="xtp", bufs=2) as xtp,
            tc.tile_pool(name="cbp", bufs=2) as cbp,
            tc.tile_pool(name="evp", bufs=4) as evp,
            tc.tile_pool(name="mmp", bufs=3, space="PSUM") as mmp,
        ):
            wt = singles.tile([128, KT, O], BF16)  # resident sign(w)^T
            s_bc = singles.tile([128, O], BF16)  # S broadcast across partitions
            b_bc = singles.tile([128, O], BF16)  # bias broadcast
            iv_sb = singles.tile([128, MC], F32)
            aa_sb = singles.tile([128, MC], F32)

            # ---- prep: tiny stat/bias loads + partition-broadcast rows ----
            nc.gpsimd.dma_start(out=iv_sb[:], in_=iv_d[:, :])
            nc.gpsimd.dma_start(out=aa_sb[:], in_=aa_d[:, :])
            sap = s_d[:]
            nc.gpsimd.dma_start(
                out=s_bc[:],
                in_=bass.AP(tensor=sap.tensor, offset=sap.offset, ap=[[0, 128]] + sap.ap),
            )
            bap = b_d[:]
            nc.gpsimd.dma_start(
                out=b_bc[:],
                in_=bass.AP(tensor=bap.tensor, offset=bap.offset, ap=[[0, 128]] + bap.ap),
            )

            # ---- prep: land fp8 sign-weight stripes, upcast into resident wt
            for k in range(KT):
                w8 = w8p.tile([128, O], FP8, tag="w8")
                nc.gpsimd.dma_start(out=w8[:], in_=w8_d[k * 128 : (k + 1) * 128, :])
                nc.vector.tensor_copy(out=wt[:, k, :], in_=w8[:])

            # ---- main: stream x^T superblocks, matmul, fused evict ----
            for s in range(NSUP):
                xtb = xtp.tile([128, KT, SUP], BF16, tag="xtb")
                xap = xt_d[:]
                nc.gpsimd.dma_start(
                    out=xtb[:],
                    in_=bass.AP(
                        tensor=xap.tensor,
                        offset=s * SUP,
                        ap=[[T, 128], [128 * T, KT], [1, SUP]],
                    ),
                )
                for c in range(CPS):
                    m = s * CPS + c
                    tsl = slice(c * 128, (c + 1) * 128)
                    # cb[p, o] = a_t * S[o] + bias[o] for this token chunk
                    cb = cbp.tile([128, O], BF16, tag="cb")
                    nc.vector.tensor_scalar_mul(cb[:], s_bc[:], aa_sb[:, m : m + 1])
                    nc.vector.tensor_add(cb[:], cb[:], b_bc[:])
                    for oc in range(NWCH):
                        osl = slice(oc * WN, (oc + 1) * WN)
                        pm = mmp.tile([128, WN], F32, tag="mm")
                        for k in range(KT):
                            nc.tensor.matmul(
                                pm[:],
                                xtb[:, k, tsl],
                                wt[:, k, osl],
                                start=(k == 0),
                                stop=(k == KT - 1),
                            )
                        ev = evp.tile([128, WN], BF16, tag="ev")
                        nc.vector.tensor_scalar_mul(ev[:], pm[:], iv_sb[:, m : m + 1])
                        nc.vector.tensor_add(ev[:], ev[:], cb[:, osl])
                        nc.gpsimd.dma_start(
                            out=out_d[m * 128 : (m + 1) * 128, osl], in_=ev[:]
                        )
    nc.compile()
    return nc


_NC_CACHE = {}
LAST_RESULTS = None


def kernel(x, weight, bias):
    global LAST_RESULTS
    x = np.asarray(x, dtype=np.float32)
    weight = np.asarray(weight, dtype=np.float32)
    bias = np.asarray(bias, dtype=np.float32)

    # LN stats folded to a per-token affine: out = (x@bw.T)*inv + a*S + b
    mu = x.mean(axis=1, dtype=np.float64)
    sd = np.sqrt(x.var(axis=1, ddof=1, dtype=np.float64))
    inv = (1.0 / (sd + EPS)).astype(np.float32)
    aa = (-mu * inv).astype(np.float32)
    iv_t = np.ascontiguousarray(inv.reshape(MC, 128).T)  # [128, chunk]
    aa_t = np.ascontiguousarray(aa.reshape(MC, 128).T)

    xT = np.ascontiguousarray(x.T).astype(NP_BF16)  # [D, T] bf16
    ws = np.sign(weight)  # [O_FULL, D] f32 in {-1, 0, +1}
    S = ws.sum(axis=1).astype(NP_BF16)  # [O_FULL]
    b16 = bias.astype(NP_BF16)
    wsT8 = ws.T.astype(NP_FP8)  # [D, O_FULL] fp8 (+-1 exact)

    if "nc" not in _NC_CACHE:
        _NC_CACHE["nc"] = _build()
    nc = _NC_CACHE["nc"]

    in_maps = []
    for i in range(NCORES):
        in_maps.append(
            {
                "xt": xT,
                "w8": wsT8[:, i * O : (i + 1) * O],
                "srow": S[i * O : (i + 1) * O],
                "brow": b16[i * O : (i + 1) * O],
                "iv": iv_t,
                "aa": aa_t,
            }
        )
    res = run_bass_kernel_spmd(nc, in_maps, list(range(NCORES)))
    LAST_RESULTS = res
    return np.concatenate(
        [res.results[i]["out"] for i in range(NCORES)], axis=1
    ).astype(np.float32)


# revision 3
# speedup vs baseline: 2.9743x; 1.4301x over previous
"""BitLinear TRN2 kernel: out = layernorm(x) @ sign(w).T + bias.

Tensor-parallel over out_features, 8 cores. Transfer-lean contract:
each core ships only its 1/8 token shard of x^T (bf16) plus its
[4096, 2048] transposed sign-weight shard (fp8); the full x^T is
assembled ON DEVICE by chunked AllGathers over NeuronLink (4 chunks,
pipelined so matmuls start after the first). Output returns as bf16
and the host concats/casts. Per-call axon traffic drops from ~2.2GB
(baseline) to ~0.64GB.

All layout/elementwise prep rides the host (sign, transposes, bf16/fp8
casts, LN stats); the device does the O(T*D*O) einsum at the PE bf16
roofline plus a 2-op DVE evict. LN is folded around the matmul on raw
x:  out[t,o] = (x@bw.T)[t,o] * inv_t + a_t * S[o] + bias[o]
with S[o] = sum_d bw[o,d], inv_t = 1/(std_t+eps), a_t = -mu_t*inv_t.
inv/a arrive per-token from the host arranged [128, 64] (partition =
token%128, col = token chunk); a_t*S[o]+bias[o] is built once per token
chunk (cb), and each PSUM eviction is psum*inv (+cb) straight to bf16.

Device layout: transposed sign weights live resident in SBUF as bf16
[128, 32, 2048] (128KB/partition), cast once from the fp8 shipment.
Gathered x^T streams in 256-token superblocks [128, 32, 256] bf16 (one
3D-AP DMA each from the gather-chunk DRAM tensor). Matmuls: stationary
= x^T k-tile [128, 128], moving = weight slice [128, 512] (one fp32
PSUM bank), 32-deep k accumulation; 8192 MMs/core ~= the 78.6 TF/s
bf16 roofline. No PE transposes, no on-device stats, everything
PE-side is bf16.
"""

import os

import numpy as np
import ml_dtypes

import concourse.bass as bass
import concourse.tile as tile
from concourse import bacc
from concourse import mybir
from concourse.bass_utils import run_bass_kernel_spmd

F32 = mybir.dt.float32
BF16 = mybir.dt.bfloat16
FP8 = mybir.dt.float8e4

NP_BF16 = ml_dtypes.bfloat16
NP_FP8 = ml_dtypes.float8_e4m3

T, D, O_FULL, NCORES = 8192, 4096, 16384, 8
O = O_FULL // NCORES  # 2048 out-features per core
TS = T // NCORES  # 1024 tokens shipped per core
EPS = 1e-5

KT = D // 128  # 32 k-tiles
MC = T // 128  # 64 token chunks
NG = 4  # x^T AllGather chunks (pipelined with compute)
GT = TS // NG  # 256 local tokens per gather chunk
SUP = 256  # tokens per superblock == GT
CPS = SUP // 128  # 2 chunks per superblock
WN = 512  # moving free width (one PSUM bank at fp32)
NWCH = O // WN  # 4 out chunks

MODE = os.environ.get("BITLIN_MODE", "bf16_ag")


def _build():
    nc = bacc.Bacc("TRN2", target_bir_lowering=False, debug=False)
    xt_d = nc.declare_dram_parameter("xt", [D, TS], BF16, isOutput=False)
    w8_d = nc.declare_dram_parameter("w8", [D, O], FP8, isOutput=False)
    s_d = nc.declare_dram_parameter("srow", [O], BF16, isOutput=False)
    b_d = nc.declare_dram_parameter("brow", [O], BF16, isOutput=False)
    iv_d = nc.declare_dram_parameter("iv", [128, MC], F32, isOutput=False)
    aa_d = nc.declare_dram_parameter("aa", [128, MC], F32, isOutput=False)
    out_d = nc.declare_dram_parameter("out", [T, O], BF16, isOutput=True)

    with tile.TileContext(nc) as tc:
        with (
            tc.tile_pool(name="dram", bufs=1, space="DRAM") as dram,
            tc.tile_pool(name="singles", bufs=1) as singles,
            tc.tile_pool(name="w8p", bufs=2) as w8p,
            tc.tile_pool(name="xtp", bufs=2) as xtp,
            tc.tile_pool(name="cbp", bufs=2) as cbp,
            tc.tile_pool(name="evp", bufs=4) as evp,
            tc.tile_pool(name="mmp", bufs=3, space="PSUM") as mmp,
        ):
            wt = singles.tile([128, KT, O], BF16)  # resident sign(w)^T
            s_bc = singles.tile([128, O], BF16)  # S broadcast across partitions
            b_bc = singles.tile([128, O], BF16)  # bias broadcast
            iv_sb = singles.tile([128, MC], F32)
            aa_sb = singles.tile([128, MC], F32)

            # ---- x^T shard -> bounce -> chunked AllGather over the 8 cores
            gxs = []
            for j in range(NG):
                ib = dram.tile([D, GT], BF16, tag=f"ib{j}", name=f"ib{j}")
                gx = dram.tile([NCORES * D, GT], BF16, tag=f"gx{j}", name=f"gx{j}")
                nc.gpsimd.dma_start(out=ib[:], in_=xt_d[:, j * GT : (j + 1) * GT])
                nc.gpsimd.collective_compute(
                    "AllGather",
                    mybir.AluOpType.bypass,
                    replica_groups=[list(range(NCORES))],
                    ins=[ib.opt()],
                    outs=[gx.opt()],
                )
                gxs.append(gx)

            # ---- prep: tiny stat/bias loads + partition-broadcast rows ----
            nc.gpsimd.dma_start(out=iv_sb[:], in_=iv_d[:, :])
            nc.gpsimd.dma_start(out=aa_sb[:], in_=aa_d[:, :])
            sap = s_d[:]
            nc.gpsimd.dma_start(
                out=s_bc[:],
                in_=bass.AP(tensor=sap.tensor, offset=sap.offset, ap=[[0, 128]] + sap.ap),
            )
            bap = b_d[:]
            nc.gpsimd.dma_start(
                out=b_bc[:],
                in_=bass.AP(tensor=bap.tensor, offset=bap.offset, ap=[[0, 128]] + bap.ap),
            )

            # ---- prep: land fp8 sign-weight stripes, upcast into resident wt
            for k in range(KT):
                w8 = w8p.tile([128, O], FP8, tag="w8")
                nc.gpsimd.dma_start(out=w8[:], in_=w8_d[k * 128 : (k + 1) * 128, :])
                nc.vector.tensor_copy(out=wt[:, k, :], in_=w8[:])

            # ---- main: stream gathered x^T superblocks, matmul, fused evict
            for j in range(NG):
                gx = gxs[j]
                for cg in range(NCORES):
                    # global tokens [cg*TS + j*GT, +SUP) live in gx rows cg*D..
                    xtb = xtp.tile([128, KT, SUP], BF16, tag="xtb")
                    gap = gx[:]
                    nc.gpsimd.dma_start(
                        out=xtb[:],
                        in_=bass.AP(
                            tensor=gap.tensor,
                            offset=gap.offset + cg * D * GT,
                            ap=[[GT, 128], [128 * GT, KT], [1, SUP]],
                        ),
                    )
                    for c in range(CPS):
                        m = (cg * TS + j * GT) // 128 + c
                        tsl = slice(c * 128, (c + 1) * 128)
                        # cb[p, o] = a_t * S[o] + bias[o] for this token chunk
                        cb = cbp.tile([128, O], BF16, tag="cb")
                        nc.vector.tensor_scalar_mul(
                            cb[:], s_bc[:], aa_sb[:, m : m + 1]
                        )
                        nc.vector.tensor_add(cb[:], cb[:], b_bc[:])
                        for oc in range(NWCH):
                            osl = slice(oc * WN, (oc + 1) * WN)
                            pm = mmp.tile([128, WN], F32, tag="mm")
                            for k in range(KT):
                                nc.tensor.matmul(
                                    pm[:],
                                    xtb[:, k, tsl],
                                    wt[:, k, osl],
                                    start=(k == 0),
                                    stop=(k == KT - 1),
                                )
                            ev = evp.tile([128, WN], BF16, tag="ev")
                            nc.vector.tensor_scalar_mul(
                                ev[:], pm[:], iv_sb[:, m : m + 1]
                            )
                            nc.vector.tensor_add(ev[:], ev[:], cb[:, osl])
                            nc.gpsimd.dma_start(
                                out=out_d[m * 128 : (m + 1) * 128, osl], in_=ev[:]
                            )
    nc.compile()
    return nc


_NC_CACHE = {}
LAST_RESULTS = None


def kernel(x, weight, bias):
    global LAST_RESULTS
    x = np.asarray(x, dtype=np.float32)
    weight = np.asarray(weight, dtype=np.float32)
    bias = np.asarray(bias, dtype=np.float32)

    # LN stats folded to a per-token affine: out = (x@bw.T)*inv + a*S + b
    mu = x.mean(axis=1, dtype=np.float64)
    sd = np.sqrt(x.var(axis=1, ddof=1, dtype=np.float64))
    inv = (1.0 / (sd + EPS)).astype(np.float32)
    aa = (-mu * inv).astype(np.float32)
    iv_t = np.ascontiguousarray(inv.reshape(MC, 128).T)  # [128, chunk]
    aa_t = np.ascontiguousarray(aa.reshape(MC, 128).T)

    xT = np.ascontiguousarray(x.T).astype(NP_BF16)  # [D, T] bf16
    ws = np.sign(weight)  # [O_FULL, D] f32 in {-1, 0, +1}
    S = ws.sum(axis=1).astype(NP_BF16)  # [O_FULL]
    b16 = bias.astype(NP_BF16)
    wsT8 = ws.T.astype(NP_FP8)  # [D, O_FULL] fp8 (+-1 exact)

    if "nc" not in _NC_CACHE:
        _NC_CACHE["nc"] = _build()
    nc = _NC_CACHE["nc"]

    in_maps = []
    for i in range(NCORES):
        in_maps.append(
            {
                "xt": xT[:, i * TS : (i + 1) * TS],
                "w8": wsT8[:, i * O : (i + 1) * O],
                "srow": S[i * O : (i + 1) * O],
                "brow": b16[i * O : (i + 1) * O],
                "iv": iv_t,
                "aa": aa_t,
            }
        )
    res = run_bass_kernel_spmd(nc, in_maps, list(range(NCORES)))
    LAST_RESULTS = res
    return np.concatenate(
        [res.results[i]["out"] for i in range(NCORES)], axis=1
    ).astype(np.float32)
